# revision 1
# baseline (speedup 1.0000x reference)
"""DeepseekV3 top-k router kernel for Trainium2 (Bass/Tile), 8-core SPMD.

Reference computation (per token, 256 experts):
  s    = sigmoid(logits)            computed as 1/(1+exp(-x)) with the DVE's
                                    bit-exact reciprocal -- bitwise identical
                                    to jax-on-neuron's logistic lowering
  s4c  = s + correction_bias
  group scores = sum of top-2 of s4c within each of 8 groups of 32
  top-4 groups -> mask -> masked s4c
  top-8 of masked s4c -> (indices, values)   [DVE Max/MaxIndex: jax tie rules]
  weights = s at those indices, normalized to sum 2.5

Sharding: data-parallel on the token dim across 8 cores (16384 tokens each);
the 256-entry correction bias is replicated. Layout: one token per SBUF
partition, its 256 expert scores along the free dim; 128 tokens per tile,
8 tiles per "supertile" so the narrow [P,8]-ish stages run batched.

Engine split: ACT does exp and the +1; the Pool (gpsimd) engine does the
bias add; DVE does the reciprocal, group maxes (the second max via a fused
custom DVE op), masking, and the Max8/FindIndex8/MatchReplace extraction.
"""
import numpy as np

import concourse.bass as bass
import concourse.bacc as bacc
import concourse.mybir as mybir
from concourse.tile import TileContext
from concourse.bass_utils import run_bass_kernel_spmd

F32 = mybir.dt.float32
U32 = mybir.dt.uint32

T_FULL = 131072
E = 256
N_CORES = 8
T_CORE = T_FULL // N_CORES      # 16384
P = 128                         # tokens per tile (one per partition)
N_TILES = T_CORE // P           # 128
B = 8                           # tiles per supertile
N_SUPER = N_TILES // B
G = 8                           # expert groups
EG = E // G                     # experts per group
BIG = 1.0e30

LAST_EXEC_NS = None
LAST_RESULTS = None

_EQNEG = None


def _get_eqneg():
    """Fused custom DVE op: out = (in0 == in1) ? -FLT_MAX : in0 (one pass,
    replaces a tensor_tensor(is_equal) + scalar_tensor_tensor pair)."""
    global _EQNEG
    if _EQNEG is None:
        from concourse.dve_ops import (DveOp, OPS, get_dve_sub_opcode,
                                       has_src1)
        from concourse.dve_spec import Spec, Src0, Src1, MaxNeg, select, eq, lower
        from concourse.dve_uop import DveOpSpec
        import concourse.dve_ops as dve_ops_mod

        spec = Spec(
            body=select(eq(Src0, Src1), MaxNeg, Src0),
            reference=lambda in0, in1, s0, s1, imm2: np.where(
                in0 == in1, np.float32(-3.4028234663852886e38), in0
            ).astype(np.float32),
        )
        op = DveOp("RTR_EQNEG", spec, subdim=False, uops_sha={})
        OPS.append(op)
        dve_ops_mod.CUSTOM_DVE_SPECS[op.name] = op.spec
        dve_ops_mod._SUB_OPCODE_FOR_NAME[op.name] = (
            dve_ops_mod._CUSTOM_DVE_ROW_BASE + len(OPS) - 1)
        assert dve_ops_mod._SUB_OPCODE_FOR_NAME[op.name] < 0x20
        for ver in ("v3", "v4"):
            tmp = DveOpSpec(name=op.name, opcode=get_dve_sub_opcode(op.name),
                            uops=lower(spec, ver=ver), rd1_en=has_src1(spec))
            op.uops_sha[ver] = tmp.sha(ver)
        _EQNEG = op
    return _EQNEG


def _build(nc: bass.Bass):
    x_d = nc.dram_tensor("logits", [T_CORE, E], F32, kind="ExternalInput")
    b_d = nc.dram_tensor("bias", [1, E], F32, kind="ExternalInput")
    idx_d = nc.dram_tensor("idx_out", [T_CORE, 8], U32, kind="ExternalOutput")
    w_d = nc.dram_tensor("w_out", [T_CORE, 8], F32, kind="ExternalOutput")

    AX = mybir.AxisListType.X
    OP = mybir.AluOpType
    ACTF = mybir.ActivationFunctionType
    eqneg = _get_eqneg()

    with TileContext(nc) as tc:
        with tc.tile_pool(name="const", bufs=1) as cpool, \
             tc.tile_pool(name="io", bufs=2) as iopool, \
             tc.tile_pool(name="wide", bufs=4) as wpool, \
             tc.tile_pool(name="keep", bufs=2) as kpool, \
             tc.tile_pool(name="slot", bufs=2) as slpool, \
             tc.tile_pool(name="small", bufs=4) as spool:

            biasb = cpool.tile([P, E], F32)
            nc.gpsimd.dma_start(out=biasb[:], in_=b_d[:, :].to_broadcast((P, E)))

            for sp_i in range(N_SUPER):
                m1s = slpool.tile([P, B * G], F32, tag="m1s")
                m2s = slpool.tile([P, B * G], F32, tag="m2s")
                gss = slpool.tile([P, B * G], F32, tag="gss")
                gts = slpool.tile([P, B * G * G], F32, tag="gts")
                ranks = slpool.tile([P, B * G], F32, tag="ranks")
                v8s = slpool.tile([P, B * 8], F32, tag="v8s")
                i8s = slpool.tile([P, B * 8], U32, tag="i8s")
                sv8s = slpool.tile([P, B * 8], F32, tag="sv8s")
                si8s = slpool.tile([P, B * 8], U32, tag="si8s")
                dens = slpool.tile([P, B], F32, tag="dens")
                rdens = slpool.tile([P, B], F32, tag="rdens")
                eqms = slpool.tile([P, B * 64], F32, tag="eqms")
                wms = slpool.tile([P, B * 64], F32, tag="wms")
                w8s = slpool.tile([P, B * 8], F32, tag="w8s")
                wouts = slpool.tile([P, B * 8], F32, tag="wouts")

                # One 1MB load per supertile. Token mapping inside the
                # supertile: partition p, tile b <-> token sp_i*1024 + 8p + b
                # (the output DMAs write the same mapping, so the DRAM
                # result is in natural token order).
                srow = sp_i * B * P
                Ls = iopool.tile([P, B * E], F32, tag="L")
                nc.sync.dma_start(
                    out=Ls[:],
                    in_=x_d[srow:srow + B * P, :].rearrange(
                        "(p x) e -> p (x e)", p=P))

                s_keep = []
                s4c_keep = []
                # ---- phase A: per tile, sigmoid / bias / group top-2 ----
                for b in range(B):
                    L = Ls[:, b * E:(b + 1) * E]

                    e = wpool.tile([P, E], F32, tag="e")
                    nc.scalar.activation(e[:], L, ACTF.Exp, scale=-1.0)
                    u = wpool.tile([P, E], F32, tag="u")
                    nc.scalar.activation(u[:], e[:], ACTF.Copy, bias=1.0)
                    s = kpool.tile([P, E], F32, tag=f"s{b}")
                    nc.vector.reciprocal(s[:], u[:])
                    s_keep.append(s)

                    s4c = kpool.tile([P, E], F32, tag=f"s4c{b}")
                    nc.gpsimd.tensor_tensor(s4c[:], s[:], biasb[:], op=OP.add)
                    s4c_keep.append(s4c)
                    s4c_g = s4c[:].rearrange("p (g e) -> p g e", g=G)

                    m1v = m1s[:, b * G:(b + 1) * G]
                    nc.vector.tensor_reduce(m1v, s4c_g, axis=AX, op=OP.max)
                    t2 = wpool.tile([P, E], F32, tag="t2")
                    nc.vector._custom_dve(
                        eqneg,
                        out=t2[:].rearrange("p (g e) -> p g e", g=G),
                        in0=s4c_g,
                        in1=m1v.unsqueeze(2).broadcast_to([P, G, EG]))
                    nc.vector.tensor_reduce(
                        m2s[:, b * G:(b + 1) * G],
                        t2[:].rearrange("p (g e) -> p g e", g=G),
                        axis=AX, op=OP.max)

                # ---- phase B: batched group ranking ----
                nc.vector.tensor_add(gss[:], m1s[:], m2s[:])
                gs3 = gss[:].rearrange("p (b g) -> p b g", b=B)
                nc.vector.tensor_tensor(
                    gts[:].rearrange("p (b i j) -> p b i j", b=B, i=G),
                    gs3.unsqueeze(2).broadcast_to([P, B, G, G]),
                    gs3.unsqueeze(3).broadcast_to([P, B, G, G]),
                    op=OP.is_gt)
                nc.vector.tensor_reduce(
                    ranks[:], gts[:].rearrange("p (b i j) -> p b i j", b=B, i=G),
                    axis=AX, op=OP.add)

                # ---- phase C: per tile top-8 extraction ----
                for b in range(B):
                    s = s_keep[b]
                    s4c = s4c_keep[b]
                    s4c_g = s4c[:].rearrange("p (g e) -> p g e", g=G)

                    masked = wpool.tile([P, E], F32, tag="masked")
                    rankv = ranks[:, b * G:(b + 1) * G]
                    nc.vector.scalar_tensor_tensor(
                        masked[:].rearrange("p (g e) -> p g e", g=G),
                        rankv.unsqueeze(2).broadcast_to([P, G, EG]), 4.0,
                        s4c_g, op0=OP.is_lt, op1=OP.mult)

                    v8 = v8s[:, b * 8:(b + 1) * 8]
                    nc.vector.max(out=v8, in_=masked[:])
                    nc.vector.max_index(i8s[:, b * 8:(b + 1) * 8], v8, masked[:])

                    marked = wpool.tile([P, E], F32, tag="marked")
                    nc.vector.match_replace(out=marked[:], in_to_replace=v8,
                                            in_values=masked[:], imm_value=BIG)
                    s_sel = wpool.tile([P, E], F32, tag="s_sel")
                    nc.vector.scalar_tensor_tensor(
                        s_sel[:], marked[:], BIG, s[:],
                        op0=OP.is_equal, op1=OP.mult,
                        accum_out=dens[:, b:b + 1])

                    sv8 = sv8s[:, b * 8:(b + 1) * 8]
                    nc.vector.max(out=sv8, in_=s_sel[:])
                    nc.vector.max_index(si8s[:, b * 8:(b + 1) * 8], sv8,
                                        s_sel[:])

                # ---- phase D: batched realign + normalize + store ----
                i8s3 = i8s[:].rearrange("p (b i) -> p b i", b=B)
                si8s3 = si8s[:].rearrange("p (b k) -> p b k", b=B)
                nc.vector.tensor_tensor(
                    eqms[:].rearrange("p (b i k) -> p b i k", b=B, i=8),
                    i8s3.unsqueeze(3).broadcast_to([P, B, 8, 8]),
                    si8s3.unsqueeze(2).broadcast_to([P, B, 8, 8]),
                    op=OP.is_equal)
                sv3 = sv8s[:].rearrange("p (b k) -> p b k", b=B)
                nc.vector.tensor_tensor(
                    wms[:].rearrange("p (b i k) -> p b i k", b=B, i=8),
                    eqms[:].rearrange("p (b i k) -> p b i k", b=B, i=8),
                    sv3.unsqueeze(2).broadcast_to([P, B, 8, 8]),
                    op=OP.mult)
                nc.vector.tensor_reduce(
                    w8s[:], wms[:].rearrange("p (b i k) -> p b i k", b=B, i=8),
                    axis=AX, op=OP.add)
                nc.vector.reciprocal(rdens[:], dens[:])
                rd3 = rdens[:].rearrange("p (b o) -> p b o", b=B)
                nc.vector.scalar_tensor_tensor(
                    wouts[:].rearrange("p (b i) -> p b i", b=B),
                    w8s[:].rearrange("p (b i) -> p b i", b=B), 2.5,
                    rd3.broadcast_to([P, B, 8]),
                    op0=OP.mult, op1=OP.mult)

                nc.sync.dma_start(
                    out=idx_d[srow:srow + B * P, :].rearrange(
                        "(p x) e -> p (x e)", p=P),
                    in_=i8s[:])
                nc.sync.dma_start(
                    out=w_d[srow:srow + B * P, :].rearrange(
                        "(p x) e -> p (x e)", p=P),
                    in_=wouts[:])
    return nc


_COMPILED_NC = None


def _get_nc():
    global _COMPILED_NC
    if _COMPILED_NC is None:
        nc = bacc.Bacc(None, target_bir_lowering=False, debug=False)
        _build(nc)
        nc.finalize()
        _COMPILED_NC = nc
    return _COMPILED_NC


def kernel(router_logits: np.ndarray, correction_bias: np.ndarray,
           trace: bool = False):
    global LAST_EXEC_NS, LAST_RESULTS
    x = np.ascontiguousarray(np.asarray(router_logits), dtype=np.float32)
    b = np.ascontiguousarray(np.asarray(correction_bias),
                             dtype=np.float32).reshape(1, E)
    assert x.shape == (T_FULL, E), x.shape

    nc = _get_nc()
    in_maps = [{"logits": x[c * T_CORE:(c + 1) * T_CORE], "bias": b}
               for c in range(N_CORES)]
    res = run_bass_kernel_spmd(nc, in_maps, core_ids=list(range(N_CORES)),
                               trace=trace)
    LAST_EXEC_NS = res.exec_time_ns
    LAST_RESULTS = res

    idx = np.concatenate([r["idx_out"] for r in res.results], axis=0)
    w = np.concatenate([r["w_out"] for r in res.results], axis=0)
    return idx.view(np.int32), w.astype(np.float32, copy=False)



# revision 2
# speedup vs baseline: 1.6049x; 1.6049x over previous
"""DeepseekV3 top-k router kernel for Trainium2 (Bass/Tile), 8-core SPMD.

v3: engine-rebalanced candidate design.

Per tile [128 tokens x 256 experts] (token-per-partition):
  ACT : s = Sigmoid(logits)                      (1 full pass)
  Pool: s4c = s + bias                           (1 full pass, TT add)
  DVE : per-group top-8 via 8x max8 on [P,32]    -> cand values g8 [P, 8*8]
  Pool: batched group-score + rank matrix TTs; cand mask TT
  DVE : v8 = max8 [P,64]; i8 = find_index8(v8, s4c full)
  DVE : marked = match_replace(v8, s4c, BIG)
  DVE : s_sel = (marked==BIG) ? s : 0 stt, accum-> dens
  DVE : sv8 = max8(s_sel); si8 = find_index8
  Pool: realign eq/mult TTs (batched); DVE: reduces + normalize smalls
"""
import numpy as np

import concourse.bass as bass
import concourse.bacc as bacc
import concourse.mybir as mybir
from concourse.tile import TileContext
from concourse.bass_utils import run_bass_kernel_spmd

F32 = mybir.dt.float32
U32 = mybir.dt.uint32

T_FULL = 131072
E = 256
N_CORES = 8
T_CORE = T_FULL // N_CORES      # 16384
P = 128                         # tokens per tile (one per partition)
N_TILES = T_CORE // P           # 128
B = 8                           # tiles per supertile
N_SUPER = N_TILES // B          # 16
G = 8                           # expert groups
EG = E // G                     # experts per group = 32
BIG = 1.0e30

LAST_EXEC_NS = None
LAST_RESULTS = None


def _build(nc: bass.Bass):
    x_d = nc.dram_tensor("logits", [T_CORE, E], F32, kind="ExternalInput")
    b_d = nc.dram_tensor("bias", [1, E], F32, kind="ExternalInput")
    idx_d = nc.dram_tensor("idx_out", [T_CORE, 8], U32, kind="ExternalOutput")
    w_d = nc.dram_tensor("w_out", [T_CORE, 8], F32, kind="ExternalOutput")

    AX = mybir.AxisListType.X
    OP = mybir.AluOpType
    ACTF = mybir.ActivationFunctionType

    with TileContext(nc) as tc:
        with tc.tile_pool(name="const", bufs=1) as cpool, \
             tc.tile_pool(name="io", bufs=2) as iopool, \
             tc.tile_pool(name="keep", bufs=2) as kpool, \
             tc.tile_pool(name="slot", bufs=2) as slpool:

            biasb = cpool.tile([P, E], F32)
            nc.gpsimd.dma_start(out=biasb[:], in_=b_d[:, :].to_broadcast((P, E)))

            for sp_i in range(N_SUPER):
                # per-supertile batched small tensors
                g8s = slpool.tile([P, B * 64], F32, tag="g8s")     # cand values
                m64s = slpool.tile([P, B * 64], F32, tag="m64s")   # masked cands
                gss = slpool.tile([P, B * G], F32, tag="gss")      # group scores
                gts = slpool.tile([P, B * G * G], F32, tag="gts")  # rank matrix
                ranks = slpool.tile([P, B * G], F32, tag="ranks")
                mask01 = slpool.tile([P, B * G], F32, tag="mask01")
                v8s = slpool.tile([P, B * 8], F32, tag="v8s")
                i8s = slpool.tile([P, B * 8], U32, tag="i8s")
                sv8s = slpool.tile([P, B * 8], F32, tag="sv8s")
                si8s = slpool.tile([P, B * 8], U32, tag="si8s")
                dens = slpool.tile([P, B], F32, tag="dens")
                rdens = slpool.tile([P, B], F32, tag="rdens")
                eqms = slpool.tile([P, B * 64], F32, tag="eqms")
                wms = slpool.tile([P, B * 64], F32, tag="wms")
                w8s = slpool.tile([P, B * 8], F32, tag="w8s")
                wouts = slpool.tile([P, B * 8], F32, tag="wouts")

                srow = sp_i * B * P
                Ls = iopool.tile([P, B * E], F32, tag="L")
                nc.sync.dma_start(
                    out=Ls[:],
                    in_=x_d[srow:srow + B * P, :].rearrange(
                        "(p x) e -> p (x e)", p=P))

                s_keep = []
                s4c_keep = []
                # ---- phase A: sigmoid (ACT), +bias (Pool), group top8 (DVE)
                for b in range(B):
                    L = Ls[:, b * E:(b + 1) * E]
                    s = kpool.tile([P, E], F32, tag=f"s{b}")
                    nc.scalar.activation(s[:], L, ACTF.Sigmoid)
                    s_keep.append(s)

                    s4c = kpool.tile([P, E], F32, tag=f"s4c{b}")
                    nc.gpsimd.tensor_tensor(s4c[:], s[:], biasb[:], op=OP.add)
                    s4c_keep.append(s4c)

                    for g in range(G):
                        nc.vector.max(
                            out=g8s[:, b * 64 + g * 8: b * 64 + g * 8 + 8],
                            in_=s4c[:, g * EG:(g + 1) * EG])

                # ---- phase B: batched group ranking ----
                # group score = top1 + top2 (first two of each group's max8)
                g84 = g8s[:].rearrange("p (b g k) -> p b g k", b=B, g=G)
                nc.gpsimd.tensor_tensor(
                    gss[:].rearrange("p (b g) -> p b g", b=B),
                    g84[:, :, :, 0], g84[:, :, :, 1], op=OP.add)
                gs3 = gss[:].rearrange("p (b g) -> p b g", b=B)
                nc.vector.tensor_tensor(
                    gts[:].rearrange("p (b i j) -> p b i j", b=B, i=G),
                    gs3.unsqueeze(2).broadcast_to([P, B, G, G]),
                    gs3.unsqueeze(3).broadcast_to([P, B, G, G]),
                    op=OP.is_gt)
                nc.vector.tensor_reduce(
                    ranks[:], gts[:].rearrange("p (b i j) -> p b i j", b=B, i=G),
                    axis=AX, op=OP.add)
                # mask01 = (rank < 4) ? 1 : 0  [P, B*G] via tensor_scalar (2x)
                nc.vector.tensor_scalar(mask01[:], ranks[:], 4.0, None,
                                        op0=OP.is_lt)
                # masked candidates: cand * mask01  [batched Pool TT, P x 512]
                nc.gpsimd.tensor_tensor(
                    m64s[:].rearrange("p (b g k) -> p b g k", b=B, g=G),
                    g84,
                    mask01[:].rearrange("p (b g) -> p b g", b=B)
                        .unsqueeze(3).broadcast_to([P, B, G, 8]),
                    op=OP.mult)

                # ---- phase C: per tile top-8 extraction ----
                for b in range(B):
                    s = s_keep[b]
                    s4c = s4c_keep[b]
                    v8 = v8s[:, b * 8:(b + 1) * 8]
                    nc.vector.max(out=v8, in_=m64s[:, b * 64:(b + 1) * 64])
                    nc.vector.max_index(i8s[:, b * 8:(b + 1) * 8], v8, s4c[:])

                    marked = kpool.tile([P, E], F32, tag=f"mk{b}")
                    nc.vector.match_replace(out=marked[:], in_to_replace=v8,
                                            in_values=s4c[:], imm_value=BIG)
                    s_sel = kpool.tile([P, E], F32, tag=f"ss{b}")
                    nc.vector.scalar_tensor_tensor(
                        s_sel[:], marked[:], BIG, s[:],
                        op0=OP.is_equal, op1=OP.mult,
                        accum_out=dens[:, b:b + 1])

                    sv8 = sv8s[:, b * 8:(b + 1) * 8]
                    nc.vector.max(out=sv8, in_=s_sel[:])
                    nc.vector.max_index(si8s[:, b * 8:(b + 1) * 8], sv8,
                                        s_sel[:])

                # ---- phase D: batched realign + normalize + store ----
                i8s3 = i8s[:].rearrange("p (b i) -> p b i", b=B)
                si8s3 = si8s[:].rearrange("p (b k) -> p b k", b=B)
                nc.vector.tensor_tensor(
                    eqms[:].rearrange("p (b i k) -> p b i k", b=B, i=8),
                    i8s3.unsqueeze(3).broadcast_to([P, B, 8, 8]),
                    si8s3.unsqueeze(2).broadcast_to([P, B, 8, 8]),
                    op=OP.is_equal)
                sv3 = sv8s[:].rearrange("p (b k) -> p b k", b=B)
                nc.gpsimd.tensor_tensor(
                    wms[:].rearrange("p (b i k) -> p b i k", b=B, i=8),
                    eqms[:].rearrange("p (b i k) -> p b i k", b=B, i=8),
                    sv3.unsqueeze(2).broadcast_to([P, B, 8, 8]),
                    op=OP.mult)
                nc.vector.tensor_reduce(
                    w8s[:], wms[:].rearrange("p (b i k) -> p b i k", b=B, i=8),
                    axis=AX, op=OP.add)
                nc.vector.reciprocal(rdens[:], dens[:])
                rd3 = rdens[:].rearrange("p (b o) -> p b o", b=B)
                nc.vector.scalar_tensor_tensor(
                    wouts[:].rearrange("p (b i) -> p b i", b=B),
                    w8s[:].rearrange("p (b i) -> p b i", b=B), 2.5,
                    rd3.broadcast_to([P, B, 8]),
                    op0=OP.mult, op1=OP.mult)

                nc.sync.dma_start(
                    out=idx_d[srow:srow + B * P, :].rearrange(
                        "(p x) e -> p (x e)", p=P),
                    in_=i8s[:])
                nc.sync.dma_start(
                    out=w_d[srow:srow + B * P, :].rearrange(
                        "(p x) e -> p (x e)", p=P),
                    in_=wouts[:])
    return nc


_COMPILED_NC = None


def _get_nc():
    global _COMPILED_NC
    if _COMPILED_NC is None:
        nc = bacc.Bacc(None, target_bir_lowering=False, debug=False)
        _build(nc)
        nc.finalize()
        _COMPILED_NC = nc
    return _COMPILED_NC


def kernel(router_logits: np.ndarray, correction_bias: np.ndarray,
           trace: bool = False):
    global LAST_EXEC_NS, LAST_RESULTS
    x = np.ascontiguousarray(np.asarray(router_logits), dtype=np.float32)
    b = np.ascontiguousarray(np.asarray(correction_bias),
                             dtype=np.float32).reshape(1, E)
    assert x.shape == (T_FULL, E), x.shape

    nc = _get_nc()
    in_maps = [{"logits": x[c * T_CORE:(c + 1) * T_CORE], "bias": b}
               for c in range(N_CORES)]
    res = run_bass_kernel_spmd(nc, in_maps, core_ids=list(range(N_CORES)),
                               trace=trace)
    LAST_EXEC_NS = res.exec_time_ns
    LAST_RESULTS = res

    idx = np.concatenate([r["idx_out"] for r in res.results], axis=0)
    w = np.concatenate([r["w_out"] for r in res.results], axis=0)
    return idx.view(np.int32), w.astype(np.float32, copy=False)
